# revision 4
# baseline (speedup 1.0000x reference)
"""Trainium2 Bass kernel for NeuronAttentionBase (dense transformer attention block).

Tensor-parallel over heads across 8 NeuronCores: each core owns 4 Q heads and
1 KV head (column-shard of Wq/Wk/Wv, row-shard of Wo), computes its partial
o_proj output; partials are summed on the host (the all-reduce step).

Per-core plan (all matmuls fp32r at full PE rate, moving dim 512):
  Phase 1: K/V projection (d-major), RoPE on K, PE-transpose of V to
           token-major.
  Phase 2: per 512-query chunk: Q projection (d-major) + RoPE, then causal
           attention in S^T layout:  S^T[t,s] = K^T.T @ Q^T chunks,
           probs = exp(S^T) (no max-subtract; scores are O(1)),
           diagonal chunks masked via precomputed 0/1 mask multiply,
           denominator via ones-stationary matmul (replicated over
           partitions), attnT = (P^T-contracted V) * recip(denom).
           attnT chunks are spilled to a DRAM scratch buffer.
  Phase 3: o_proj: out[tok, hid] = sum_h attnT_h.T @ Wo_h, streamed from the
           DRAM scratch, partial written to DRAM.
"""

import sys
import math
from contextlib import ExitStack

import numpy as np

sys.path.insert(0, "/opt/trn_rl_repo")

B, S, HID = 2, 2048, 4096
NH, NKV, D = 32, 8, 128
NCORES = 8
HQ = NH // NCORES            # 4 q heads per core
TOK = B * S                  # 4096 flattened tokens
SC = 512                     # s-chunk (query block)
NKC = HID // 128             # 32 contraction chunks
NSC = S // SC                # 4 s-chunks per batch
NJT = S // 128               # 16 t-tiles per batch

_RUNNERS = {}


def _phase1(nc, tc, ctx, env):
    """K/V projection + RoPE(K) + V transpose for both batches."""
    mybir = env["mybir"]
    F32, F32R = mybir.dt.float32, mybir.dt.float32r
    MUL, ADD = mybir.AluOpType.mult, mybir.AluOpType.add
    hT, cosT, sinR = env["hT"], env["cosT"], env["sinR"]
    wk, wv = env["wk"], env["wv"]
    rotm_t, ident_t = env["rotm_t"], env["ident_t"]
    kt_b, vtm_b = env["kt_b"], env["vtm_b"]

    wkv = ctx.enter_context(tc.tile_pool(name="wkv", bufs=1))
    ht1 = ctx.enter_context(tc.tile_pool(name="ht1", bufs=2))
    cs1 = ctx.enter_context(tc.tile_pool(name="cs1", bufs=2))
    tmp1 = ctx.enter_context(tc.tile_pool(name="tmp1", bufs=2))
    vts = ctx.enter_context(tc.tile_pool(name="vts", bufs=1))
    kvps = ctx.enter_context(tc.tile_pool(name="kvps", bufs=8, space="PSUM"))

    wk_all = wkv.tile([128, NKC * 128], F32R, tag="wk")
    wv_all = wkv.tile([128, NKC * 128], F32R, tag="wv")
    nc.sync.dma_start(
        wk_all[:].rearrange("p (kk c) -> p kk c", c=128),
        wk[:].bitcast(F32R).rearrange("(kk p) c -> p kk c", p=128))
    nc.sync.dma_start(
        wv_all[:].rearrange("p (kk c) -> p kk c", c=128),
        wv[:].bitcast(F32R).rearrange("(kk p) c -> p kk c", p=128))

    for b in range(B):
        t0 = b * S
        ktp = [kvps.tile([128, SC], F32, tag="kv", name=f"ktp{b}_{i}") for i in range(4)]
        vtp = [kvps.tile([128, SC], F32, tag="kv", name=f"vtp{b}_{i}") for i in range(4)]
        for k in range(NKC):
            ht = ht1.tile([128, S], F32R, tag="ht")
            nc.sync.dma_start(
                ht[:], hT[128 * k:128 * (k + 1), t0:t0 + S].bitcast(F32R))
            for c in range(4):
                nc.tensor.matmul(
                    ktp[c][:], wk_all[:, 128 * k:128 * (k + 1)],
                    ht[:, SC * c:SC * (c + 1)],
                    start=(k == 0), stop=(k == NKC - 1))
                nc.tensor.matmul(
                    vtp[c][:], wv_all[:, 128 * k:128 * (k + 1)],
                    ht[:, SC * c:SC * (c + 1)],
                    start=(k == 0), stop=(k == NKC - 1))
        # V^T psum -> sbuf staging (frees 4 psum banks)
        vt_stage = vts.tile([128, S], F32, tag="vts")
        for c in range(4):
            nc.vector.tensor_copy(vt_stage[:, SC * c:SC * (c + 1)], vtp[c][:])
        # RoPE on K, per 512-chunk
        for c in range(4):
            cs = cs1.tile([128, SC], F32, tag="cs")
            sn = cs1.tile([128, SC], F32, tag="sn")
            nc.sync.dma_start(cs[:], cosT[:, t0 + SC * c:t0 + SC * (c + 1)])
            nc.sync.dma_start(sn[:], sinR[:, t0 + SC * c:t0 + SC * (c + 1)])
            y = tmp1.tile([128, SC], F32R, tag="y")
            nc.vector.tensor_tensor(out=y[:], in0=ktp[c][:], in1=sn[:], op=MUL)
            roty = kvps.tile([128, SC], F32, tag="kv")
            nc.tensor.matmul(roty[:], rotm_t[:], y[:], start=True, stop=True)
            ta = tmp1.tile([128, SC], F32, tag="ta")
            nc.vector.tensor_tensor(out=ta[:], in0=ktp[c][:], in1=cs[:], op=MUL)
            nc.vector.tensor_tensor(
                out=kt_b[b][:, SC * c:SC * (c + 1)], in0=ta[:], in1=roty[:], op=ADD)
        # V transpose: 16 PE transposes -> token-major Vtm
        for j in range(NJT):
            pvt = kvps.tile([128, 128], F32, tag="kv")
            nc.tensor.transpose(pvt[:], vt_stage[:, 128 * j:128 * (j + 1)], ident_t[:])
            nc.vector.tensor_copy(vtm_b[b][:, 128 * j:128 * (j + 1)], pvt[:])


def _qproj_rope(nc, pools, env, b, kappa):
    """Project 4 Q heads for one 512-token chunk and apply RoPE. Returns qt list."""
    mybir = env["mybir"]
    F32, F32R = mybir.dt.float32, mybir.dt.float32r
    MUL, ADD = mybir.AluOpType.mult, mybir.AluOpType.add
    hT, cosT, sinR, wq_all = env["hT"], env["cosT"], env["sinR"], env["wq_all"]
    rotm_t = env["rotm_t"]
    qps, scps, ht2, cs2, tmp2, qtp = (pools[k] for k in
                                      ("qps", "scps", "ht2", "cs2", "tmp2", "qtp"))
    t0 = b * S + SC * kappa
    qA = qps.tile([128, 1024], F32, tag="q")
    qB = qps.tile([128, 1024], F32, tag="q")
    for k in range(NKC):
        ht = ht2.tile([128, SC], F32R, tag="ht")
        nc.sync.dma_start(
            ht[:], hT[128 * k:128 * (k + 1), t0:t0 + SC].bitcast(F32R))
        for h in range(HQ):
            dst = (qA if h < 2 else qB)
            col = 512 * (h % 2)
            nc.tensor.matmul(
                dst[:, col:col + 512],
                env["wq_all"][:, 512 * k + 128 * h:512 * k + 128 * (h + 1)],
                ht[:], start=(k == 0), stop=(k == NKC - 1))
    cs = cs2.tile([128, SC], F32, tag="cs")
    sn = cs2.tile([128, SC], F32, tag="sn")
    nc.sync.dma_start(cs[:], cosT[:, t0:t0 + SC])
    nc.sync.dma_start(sn[:], sinR[:, t0:t0 + SC])
    qt = []
    for h in range(HQ):
        src = (qA if h < 2 else qB)
        qsl = src[:, 512 * (h % 2):512 * (h % 2) + 512]
        y = tmp2.tile([128, SC], F32R, tag="y")
        nc.vector.tensor_tensor(out=y[:], in0=qsl, in1=sn[:], op=MUL)
        roty = scps.tile([128, 1024], F32, tag="sc")
        nc.tensor.matmul(roty[:, 0:512], rotm_t[:], y[:], start=True, stop=True)
        ta = tmp2.tile([128, SC], F32, tag="ta")
        nc.vector.tensor_tensor(out=ta[:], in0=qsl, in1=cs[:], op=MUL)
        qh = qtp.tile([128, SC], F32R, tag="qt")
        nc.vector.tensor_tensor(out=qh[:], in0=ta[:], in1=roty[:, 0:512], op=ADD)
        qt.append(qh)
    return qt


def _attn_head(nc, pools, env, mode, b, kappa, h, qh):
    """Attention for one (batch, s-chunk, head): probs, denom, PV, normalize, spill."""
    mybir = env["mybir"]
    F32, F32R = mybir.dt.float32, mybir.dt.float32r
    MUL, ADD = mybir.AluOpType.mult, mybir.AluOpType.add
    EXP = mybir.ActivationFunctionType.Exp
    kt_b, vtm_b = env["kt_b"], env["vtm_b"]
    ones_t, mbig_t, attnT_d = env["ones_t"], env["mbig_t"], env["attnT_d"]
    scps, atps, dnps, prb, ans, rcp, bia = (pools[k] for k in
        ("scps", "atps", "dnps", "prb", "ans", "rcp", "bia"))
    t0 = b * S + SC * kappa
    jm = 4 * kappa + 4 if mode == "causal" else NJT

    probs = prb.tile([128, jm * 512], F32R, tag="probs")
    for jp in range(jm // 2):
        j0, j1 = 2 * jp, 2 * jp + 1
        sc_ps = scps.tile([128, 1024], F32, tag="sc")
        nc.tensor.matmul(sc_ps[:, 0:512],
                         kt_b[b][:, 128 * j0:128 * (j0 + 1)], qh[:],
                         start=True, stop=True)
        nc.tensor.matmul(sc_ps[:, 512:1024],
                         kt_b[b][:, 128 * j1:128 * (j1 + 1)], qh[:],
                         start=True, stop=True)
        if mode == "bias":
            for jj in range(2):
                j = 2 * jp + jj
                bt = bia.tile([128, SC], F32, tag="bias")
                nc.sync.dma_start(
                    bt[:], env["biasT"][b, 128 * j:128 * (j + 1),
                                        SC * kappa:SC * (kappa + 1)])
                nc.vector.tensor_tensor(
                    out=sc_ps[:, 512 * jj:512 * (jj + 1)],
                    in0=sc_ps[:, 512 * jj:512 * (jj + 1)], in1=bt[:], op=ADD)
        nc.scalar.activation(probs[:, 1024 * jp:1024 * (jp + 1)], sc_ps[:], EXP)
    if mode == "causal":
        for j in range(4 * kappa, 4 * kappa + 4):
            off = 128 * j - 512 * kappa  # 0,128,256,384
            msl = mbig_t[:, 384 - off:384 - off + 512]
            nc.vector.tensor_tensor(
                out=probs[:, 512 * j:512 * (j + 1)],
                in0=probs[:, 512 * j:512 * (j + 1)], in1=msl, op=MUL)
    den = dnps.tile([128, SC], F32, tag="den")
    for j in range(jm):
        nc.tensor.matmul(den[:], ones_t[:], probs[:, 512 * j:512 * (j + 1)],
                         start=(j == 0), stop=(j == jm - 1))
    rec = rcp.tile([128, SC], F32, tag="rec")
    nc.vector.reciprocal_approx_fast(out=rec[:], in_=den[:])
    at = atps.tile([128, SC], F32, tag="at")
    for j in range(jm):
        nc.tensor.matmul(at[:], vtm_b[b][:, 128 * j:128 * (j + 1)],
                         probs[:, 512 * j:512 * (j + 1)],
                         start=(j == 0), stop=(j == jm - 1))
    atn = ans.tile([128, SC], F32R, tag="atn")
    nc.vector.tensor_tensor(out=atn[:], in0=at[:], in1=rec[:], op=MUL)
    nc.sync.dma_start(attnT_d[128 * h:128 * (h + 1), t0:t0 + SC],
                      atn[:].bitcast(F32))


def _phase2(nc, tc, ctx, env, mode):
    mybir = env["mybir"]
    F32R = mybir.dt.float32r
    pools = {}
    pools["wqp"] = ctx.enter_context(tc.tile_pool(name="wqp", bufs=1))
    pools["ht2"] = ctx.enter_context(tc.tile_pool(name="ht2", bufs=4))
    pools["cs2"] = ctx.enter_context(tc.tile_pool(name="cs2", bufs=2))
    pools["tmp2"] = ctx.enter_context(tc.tile_pool(name="tmp2", bufs=2))
    pools["qtp"] = ctx.enter_context(tc.tile_pool(name="qtp", bufs=6))
    pools["prb"] = ctx.enter_context(tc.tile_pool(name="prb", bufs=1))
    pools["ans"] = ctx.enter_context(tc.tile_pool(name="ans", bufs=3))
    pools["rcp"] = ctx.enter_context(tc.tile_pool(name="rcp", bufs=2))
    pools["bia"] = ctx.enter_context(tc.tile_pool(name="bia", bufs=2))
    pools["qps"] = ctx.enter_context(tc.tile_pool(name="qps", bufs=2, space="PSUM"))
    pools["scps"] = ctx.enter_context(tc.tile_pool(name="scps", bufs=1, space="PSUM"))
    pools["atps"] = ctx.enter_context(tc.tile_pool(name="atps", bufs=1, space="PSUM"))
    pools["dnps"] = ctx.enter_context(tc.tile_pool(name="dnps", bufs=1, space="PSUM"))

    wq_all = pools["wqp"].tile([128, NKC * 512], F32R, tag="wq")
    nc.sync.dma_start(
        wq_all[:].rearrange("p (kk c) -> p kk c", c=512),
        env["wq"][:].bitcast(F32R).rearrange("(kk p) c -> p kk c", p=128))
    env["wq_all"] = wq_all

    for b in range(B):
        for kappa in range(NSC):
            qt = _qproj_rope(nc, pools, env, b, kappa)
            for h in range(HQ):
                _attn_head(nc, pools, env, mode, b, kappa, h, qt[h])


def _phase3(nc, tc, ctx, env):
    mybir = env["mybir"]
    F32, F32R = mybir.dt.float32, mybir.dt.float32r
    wo, attnT_d, out = env["wo"], env["attnT_d"], env["out"]
    wop = ctx.enter_context(tc.tile_pool(name="wop", bufs=1))
    atl = ctx.enter_context(tc.tile_pool(name="atl", bufs=2))
    osb = ctx.enter_context(tc.tile_pool(name="osb", bufs=4))
    ops = ctx.enter_context(tc.tile_pool(name="ops", bufs=6, space="PSUM"))

    wo_all = wop.tile([128, HQ * HID], F32R, tag="wo")
    nc.sync.dma_start(
        wo_all[:].rearrange("p (h c) -> p h c", c=HID),
        wo[:].bitcast(F32R).rearrange("(h p) c -> p h c", p=128))
    for g in range(TOK // SC):
        a_h = []
        for h in range(HQ):
            a = atl.tile([128, SC], F32R, tag=f"a{h}")
            nc.sync.dma_start(
                a[:], attnT_d[128 * h:128 * (h + 1),
                              SC * g:SC * (g + 1)].bitcast(F32R))
            a_h.append(a)
        for m in range(SC // 128):
            for n in range(HID // 512):
                ps = ops.tile([128, 512], F32, tag="o")
                for h in range(HQ):
                    nc.tensor.matmul(
                        ps[:], a_h[h][:, 128 * m:128 * (m + 1)],
                        wo_all[:, HID * h + 512 * n:HID * h + 512 * (n + 1)],
                        start=(h == 0), stop=(h == HQ - 1))
                ob = osb.tile([128, 512], F32, tag="ob")
                nc.any.tensor_copy(ob[:], ps[:])
                nc.sync.dma_start(
                    out[SC * g + 128 * m:SC * g + 128 * (m + 1),
                        512 * n:512 * (n + 1)], ob[:])


def _build_nc(mode):
    """mode in {"causal", "full", "bias"}."""
    import concourse.bass as bass  # noqa: F401
    import concourse.mybir as mybir
    import concourse.tile as tile
    from concourse import bacc

    F32 = mybir.dt.float32
    F32R = mybir.dt.float32r

    nc = bacc.Bacc("TRN2", target_bir_lowering=False)

    env = {"mybir": mybir}
    env["hT"] = nc.dram_tensor("hT", [HID, TOK], F32, kind="ExternalInput")
    env["wq"] = nc.dram_tensor("wq", [HID, HQ * D], F32, kind="ExternalInput")
    env["wk"] = nc.dram_tensor("wk", [HID, D], F32, kind="ExternalInput")
    env["wv"] = nc.dram_tensor("wv", [HID, D], F32, kind="ExternalInput")
    env["wo"] = nc.dram_tensor("wo", [HQ * D, HID], F32, kind="ExternalInput")
    env["cosT"] = nc.dram_tensor("cosT", [D, TOK], F32, kind="ExternalInput")
    env["sinR"] = nc.dram_tensor("sinR", [D, TOK], F32, kind="ExternalInput")
    rotm = nc.dram_tensor("rotm", [128, 128], F32, kind="ExternalInput")
    ident = nc.dram_tensor("ident", [128, 128], F32, kind="ExternalInput")
    ones = nc.dram_tensor("ones", [128, 128], F32, kind="ExternalInput")
    mbig = nc.dram_tensor("mbig", [128, 896], F32, kind="ExternalInput")
    if mode == "bias":
        env["biasT"] = nc.dram_tensor("biasT", [B, S, S], F32, kind="ExternalInput")
    env["out"] = nc.dram_tensor("out", [TOK, HID], F32, kind="ExternalOutput")

    with tile.TileContext(nc) as tc, ExitStack() as ctx:
        cpool = ctx.enter_context(tc.tile_pool(name="consts", bufs=1))
        kvsb = ctx.enter_context(tc.tile_pool(name="kvsb", bufs=1))
        adp = ctx.enter_context(tc.tile_pool(name="adram", bufs=1, space="DRAM"))

        env["rotm_t"] = cpool.tile([128, 128], F32R, tag="rotm", name="rotm_t")
        env["ident_t"] = cpool.tile([128, 128], F32, tag="ident", name="ident_t")
        env["ones_t"] = cpool.tile([128, 128], F32R, tag="ones", name="ones_t")
        env["mbig_t"] = cpool.tile([128, 896], F32, tag="mbig", name="mbig_t")
        nc.sync.dma_start(env["rotm_t"][:], rotm[:].bitcast(F32R))
        nc.sync.dma_start(env["ident_t"][:], ident[:])
        nc.sync.dma_start(env["ones_t"][:], ones[:].bitcast(F32R))
        nc.sync.dma_start(env["mbig_t"][:], mbig[:])

        env["attnT_d"] = adp.tile([HQ * D, TOK], F32, tag="attnTd", name="attnT_d")
        env["kt_b"] = [kvsb.tile([128, S], F32R, tag=f"ktb{b}", name=f"kt_b{b}") for b in range(B)]
        env["vtm_b"] = [kvsb.tile([128, S], F32R, tag=f"vtmb{b}", name=f"vtm_b{b}") for b in range(B)]

        with ExitStack() as p1ctx:
            _phase1(nc, tc, p1ctx, env)
        with ExitStack() as p2ctx:
            _phase2(nc, tc, p2ctx, env, mode)
        with ExitStack() as p3ctx:
            _phase3(nc, tc, p3ctx, env)
    nc.finalize()
    return nc


def _get_runner(mode):
    if mode in _RUNNERS:
        return _RUNNERS[mode]
    nc = _build_nc(mode)
    _RUNNERS[mode] = nc
    return nc


def _host_prep(hidden_states, Wq, Wk, Wv, Wo, cos_cache, sin_cache,
               position_ids, attention_mask):
    hidden_states = np.asarray(hidden_states, dtype=np.float32)
    Wq = np.asarray(Wq, dtype=np.float32)
    Wk = np.asarray(Wk, dtype=np.float32)
    Wv = np.asarray(Wv, dtype=np.float32)
    Wo = np.asarray(Wo, dtype=np.float32)
    cos_cache = np.asarray(cos_cache, dtype=np.float32)
    sin_cache = np.asarray(sin_cache, dtype=np.float32)
    position_ids = np.asarray(position_ids)
    mask = np.asarray(attention_mask)

    hT = np.ascontiguousarray(hidden_states.reshape(TOK, HID).T)
    cos_g = cos_cache[position_ids.astype(np.int64)]   # [B, S, D]
    sin_g = sin_cache[position_ids.astype(np.int64)]
    cosT = np.ascontiguousarray(cos_g.reshape(TOK, D).T)          # [D, TOK]
    sinT = np.ascontiguousarray(sin_g.reshape(TOK, D).T)
    sinR = np.ascontiguousarray(np.roll(sinT, -64, axis=0))       # w[d]=sin[(d+64)%128]

    m2 = mask.reshape(B, S, S)
    tril = np.tril(np.ones((S, S), dtype=bool))
    if all(np.array_equal(m2[b], tril) for b in range(B)):
        mode = "causal"
    elif m2.all():
        mode = "full"
    else:
        mode = "bias"

    rotm = np.zeros((128, 128), dtype=np.float32)
    for i in range(64):
        rotm[64 + i, i] = -1.0
        rotm[i, 64 + i] = 1.0
    identm = np.eye(128, dtype=np.float32)
    onesm = np.ones((128, 128), dtype=np.float32)
    # Mbig[p, y] = 1 iff y >= p + 384  (slices give the 4 diagonal masks)
    yy = np.arange(896)[None, :]
    pp = np.arange(128)[:, None]
    mbig = (yy >= pp + 384).astype(np.float32)

    scale = np.float32(1.0 / math.sqrt(D))
    common = dict(hT=hT, cosT=cosT, sinR=sinR, rotm=rotm, ident=identm,
                  ones=onesm, mbig=mbig)
    if mode == "bias":
        biasT = np.where(m2, np.float32(0), np.float32(-1e30)).astype(np.float32)
        biasT = np.ascontiguousarray(biasT.transpose(0, 2, 1))  # [B, t, s]
        common["biasT"] = biasT

    in_maps = []
    for c in range(NCORES):
        m = dict(common)
        m["wq"] = np.ascontiguousarray(Wq[:, c * HQ * D:(c + 1) * HQ * D] * scale)
        m["wk"] = np.ascontiguousarray(Wk[:, c * D:(c + 1) * D])
        m["wv"] = np.ascontiguousarray(Wv[:, c * D:(c + 1) * D])
        m["wo"] = np.ascontiguousarray(Wo[c * HQ * D:(c + 1) * HQ * D, :])
        in_maps.append(m)
    return mode, in_maps


def kernel(hidden_states, Wq, Wk, Wv, Wo, cos_cache, sin_cache,
           position_ids, attention_mask):
    from concourse.bass_utils import run_bass_kernel_spmd

    mode, in_maps = _host_prep(hidden_states, Wq, Wk, Wv, Wo, cos_cache,
                               sin_cache, position_ids, attention_mask)
    nc = _get_runner(mode)
    res = run_bass_kernel_spmd(nc, in_maps, core_ids=list(range(NCORES)),
                               trace=False)
    acc = np.zeros((TOK, HID), dtype=np.float32)
    for c in range(NCORES):
        acc += res.results[c]["out"]
    return acc.reshape(B, S, HID)


# revision 6
# speedup vs baseline: 1347.3100x; 1347.3100x over previous
"""Trainium2 Bass kernel for NeuronAttentionBase (dense transformer attention block).

Tensor-parallel over heads across 8 NeuronCores: each core owns 4 Q heads and
1 KV head (column-shard of Wq/Wk/Wv, row-shard of Wo), computes its partial
o_proj output; partials are summed on the host (the all-reduce step).

Per-core plan (all matmuls fp32r at full PE rate, moving dim 512):
  Phase 1: K/V projection (d-major), RoPE on K, PE-transpose of V to
           token-major.
  Phase 2: per 512-query chunk: Q projection (d-major) + RoPE, then causal
           attention in S^T layout:  S^T[t,s] = K^T.T @ Q^T chunks,
           probs = exp(S^T) (no max-subtract; scores are O(1)),
           diagonal chunks masked via precomputed 0/1 mask multiply,
           denominator via ones-stationary matmul (replicated over
           partitions), attnT = (P^T-contracted V) * recip(denom).
           attnT chunks are spilled to a DRAM scratch buffer.
  Phase 3: o_proj: out[tok, hid] = sum_h attnT_h.T @ Wo_h, streamed from the
           DRAM scratch, partial written to DRAM.
"""

import sys
import math
from contextlib import ExitStack

import numpy as np

sys.path.insert(0, "/opt/trn_rl_repo")

B, S, HID = 2, 2048, 4096
NH, NKV, D = 32, 8, 128
NCORES = 8
HQ = NH // NCORES            # 4 q heads per core
TOK = B * S                  # 4096 flattened tokens
SC = 512                     # s-chunk (query block)
NKC = HID // 128             # 32 contraction chunks
NSC = S // SC                # 4 s-chunks per batch
NJT = S // 128               # 16 t-tiles per batch

_RUNNERS = {}


def _phase1(nc, tc, ctx, env):
    """K/V projection + RoPE(K) + V transpose for both batches."""
    mybir = env["mybir"]
    F32, F32R = mybir.dt.float32, mybir.dt.float32r
    MUL, ADD = mybir.AluOpType.mult, mybir.AluOpType.add
    hT, cosT, sinR = env["hT"], env["cosT"], env["sinR"]
    wk, wv = env["wk"], env["wv"]
    rotm_t, ident_t = env["rotm_t"], env["ident_t"]
    kt_b, vtm_b = env["kt_b"], env["vtm_b"]

    wkv = ctx.enter_context(tc.tile_pool(name="wkv", bufs=1))
    ht1 = ctx.enter_context(tc.tile_pool(name="ht1", bufs=2))
    cs1 = ctx.enter_context(tc.tile_pool(name="cs1", bufs=2))
    tmp1 = ctx.enter_context(tc.tile_pool(name="tmp1", bufs=2))
    vts = ctx.enter_context(tc.tile_pool(name="vts", bufs=1))
    kvps = ctx.enter_context(tc.tile_pool(name="kvps", bufs=8, space="PSUM"))

    wk_all = wkv.tile([128, NKC * 128], F32R, tag="wk")
    wv_all = wkv.tile([128, NKC * 128], F32R, tag="wv")
    nc.sync.dma_start(
        wk_all[:].rearrange("p (kk c) -> p kk c", c=128),
        wk[:].bitcast(F32R).rearrange("(kk p) c -> p kk c", p=128))
    nc.sync.dma_start(
        wv_all[:].rearrange("p (kk c) -> p kk c", c=128),
        wv[:].bitcast(F32R).rearrange("(kk p) c -> p kk c", p=128))

    for b in range(B):
        t0 = b * S
        ktp = [kvps.tile([128, SC], F32, tag="kv", name=f"ktp{b}_{i}") for i in range(4)]
        vtp = [kvps.tile([128, SC], F32, tag="kv", name=f"vtp{b}_{i}") for i in range(4)]
        for k in range(NKC):
            ht = ht1.tile([128, S], F32R, tag="ht")
            nc.sync.dma_start(
                ht[:], hT[128 * k:128 * (k + 1), t0:t0 + S].bitcast(F32R))
            for c in range(4):
                nc.tensor.matmul(
                    ktp[c][:], wk_all[:, 128 * k:128 * (k + 1)],
                    ht[:, SC * c:SC * (c + 1)],
                    start=(k == 0), stop=(k == NKC - 1))
                nc.tensor.matmul(
                    vtp[c][:], wv_all[:, 128 * k:128 * (k + 1)],
                    ht[:, SC * c:SC * (c + 1)],
                    start=(k == 0), stop=(k == NKC - 1))
        # V^T psum -> sbuf staging (frees 4 psum banks)
        vt_stage = vts.tile([128, S], F32, tag="vts")
        for c in range(4):
            nc.vector.tensor_copy(vt_stage[:, SC * c:SC * (c + 1)], vtp[c][:])
        # RoPE on K, per 512-chunk
        for c in range(4):
            cs = cs1.tile([128, SC], F32, tag="cs")
            sn = cs1.tile([128, SC], F32, tag="sn")
            nc.sync.dma_start(cs[:], cosT[:, t0 + SC * c:t0 + SC * (c + 1)])
            nc.sync.dma_start(sn[:], sinR[:, t0 + SC * c:t0 + SC * (c + 1)])
            y = tmp1.tile([128, SC], F32R, tag="y")
            nc.vector.tensor_tensor(out=y[:], in0=ktp[c][:], in1=sn[:], op=MUL)
            roty = kvps.tile([128, SC], F32, tag="kv")
            nc.tensor.matmul(roty[:], rotm_t[:], y[:], start=True, stop=True)
            ta = tmp1.tile([128, SC], F32, tag="ta")
            nc.vector.tensor_tensor(out=ta[:], in0=ktp[c][:], in1=cs[:], op=MUL)
            nc.vector.tensor_tensor(
                out=kt_b[b][:, SC * c:SC * (c + 1)], in0=ta[:], in1=roty[:], op=ADD)
        # V transpose: 16 PE transposes -> token-major Vtm
        for j in range(NJT):
            pvt = kvps.tile([128, 128], F32, tag="kv")
            nc.tensor.transpose(pvt[:], vt_stage[:, 128 * j:128 * (j + 1)], ident_t[:])
            nc.vector.tensor_copy(vtm_b[b][:, 128 * j:128 * (j + 1)], pvt[:])


def _qproj_rope(nc, pools, env, b, kappa):
    """Project 4 Q heads for one 512-token chunk and apply RoPE. Returns qt list."""
    mybir = env["mybir"]
    F32, F32R = mybir.dt.float32, mybir.dt.float32r
    MUL, ADD = mybir.AluOpType.mult, mybir.AluOpType.add
    hT, cosT, sinR, wq_all = env["hT"], env["cosT"], env["sinR"], env["wq_all"]
    rotm_t = env["rotm_t"]
    qps, scps, ht2, cs2, tmp2, qtp = (pools[k] for k in
                                      ("qps", "scps", "ht2", "cs2", "tmp2", "qtp"))
    t0 = b * S + SC * kappa
    qA = qps.tile([128, 1024], F32, tag="q")
    qB = qps.tile([128, 1024], F32, tag="q")
    for k in range(NKC):
        ht = ht2.tile([128, SC], F32R, tag="ht")
        nc.sync.dma_start(
            ht[:], hT[128 * k:128 * (k + 1), t0:t0 + SC].bitcast(F32R))
        for h in range(HQ):
            dst = (qA if h < 2 else qB)
            col = 512 * (h % 2)
            nc.tensor.matmul(
                dst[:, col:col + 512],
                env["wq_all"][:, 512 * k + 128 * h:512 * k + 128 * (h + 1)],
                ht[:], start=(k == 0), stop=(k == NKC - 1))
    cs = cs2.tile([128, SC], F32, tag="cs")
    sn = cs2.tile([128, SC], F32, tag="sn")
    nc.sync.dma_start(cs[:], cosT[:, t0:t0 + SC])
    nc.sync.dma_start(sn[:], sinR[:, t0:t0 + SC])
    qt = []
    for h in range(HQ):
        src = (qA if h < 2 else qB)
        qsl = src[:, 512 * (h % 2):512 * (h % 2) + 512]
        y = tmp2.tile([128, SC], F32R, tag="y")
        nc.vector.tensor_tensor(out=y[:], in0=qsl, in1=sn[:], op=MUL)
        roty = scps.tile([128, 1024], F32, tag="sc")
        nc.tensor.matmul(roty[:, 0:512], rotm_t[:], y[:], start=True, stop=True)
        ta = tmp2.tile([128, SC], F32, tag="ta")
        nc.vector.tensor_tensor(out=ta[:], in0=qsl, in1=cs[:], op=MUL)
        qh = qtp.tile([128, SC], F32R, tag="qt")
        nc.vector.tensor_tensor(out=qh[:], in0=ta[:], in1=roty[:, 0:512], op=ADD)
        qt.append(qh)
    return qt


def _attn_head(nc, pools, env, mode, b, kappa, h, qh):
    """Attention for one (batch, s-chunk, head): probs, denom, PV, normalize, spill."""
    mybir = env["mybir"]
    F32, F32R = mybir.dt.float32, mybir.dt.float32r
    MUL, ADD = mybir.AluOpType.mult, mybir.AluOpType.add
    EXP = mybir.ActivationFunctionType.Exp
    kt_b, vtm_b = env["kt_b"], env["vtm_b"]
    ones_t, mbig_t, attnT_d = env["ones_t"], env["mbig_t"], env["attnT_d"]
    scps, atps, dnps, prb, ans, rcp, bia = (pools[k] for k in
        ("scps", "atps", "dnps", "prb", "ans", "rcp", "bia"))
    t0 = b * S + SC * kappa
    jm = 4 * kappa + 4 if mode == "causal" else NJT

    probs = prb.tile([128, jm * 512], F32R, tag="probs")
    for jp in range(jm // 2):
        j0, j1 = 2 * jp, 2 * jp + 1
        sc_ps = scps.tile([128, 1024], F32, tag="sc")
        nc.tensor.matmul(sc_ps[:, 0:512],
                         kt_b[b][:, 128 * j0:128 * (j0 + 1)], qh[:],
                         start=True, stop=True)
        nc.tensor.matmul(sc_ps[:, 512:1024],
                         kt_b[b][:, 128 * j1:128 * (j1 + 1)], qh[:],
                         start=True, stop=True)
        if mode == "bias":
            for jj in range(2):
                j = 2 * jp + jj
                bt = bia.tile([128, SC], F32, tag="bias")
                nc.sync.dma_start(
                    bt[:], env["biasT"][b, 128 * j:128 * (j + 1),
                                        SC * kappa:SC * (kappa + 1)])
                nc.vector.tensor_tensor(
                    out=sc_ps[:, 512 * jj:512 * (jj + 1)],
                    in0=sc_ps[:, 512 * jj:512 * (jj + 1)], in1=bt[:], op=ADD)
        nc.scalar.activation(probs[:, 1024 * jp:1024 * (jp + 1)], sc_ps[:], EXP)
    if mode == "causal":
        for j in range(4 * kappa, 4 * kappa + 4):
            off = 128 * j - 512 * kappa  # 0,128,256,384
            msl = mbig_t[:, 384 - off:384 - off + 512]
            nc.vector.tensor_tensor(
                out=probs[:, 512 * j:512 * (j + 1)],
                in0=probs[:, 512 * j:512 * (j + 1)], in1=msl, op=MUL)
    den = dnps.tile([128, SC], F32, tag="den")
    for j in range(jm):
        nc.tensor.matmul(den[:], ones_t[:], probs[:, 512 * j:512 * (j + 1)],
                         start=(j == 0), stop=(j == jm - 1))
    rec = rcp.tile([128, SC], F32, tag="rec")
    nc.vector.reciprocal_approx_fast(out=rec[:], in_=den[:])
    at = atps.tile([128, SC], F32, tag="at")
    for j in range(jm):
        nc.tensor.matmul(at[:], vtm_b[b][:, 128 * j:128 * (j + 1)],
                         probs[:, 512 * j:512 * (j + 1)],
                         start=(j == 0), stop=(j == jm - 1))
    atn = ans.tile([128, SC], F32R, tag="atn")
    nc.vector.tensor_tensor(out=atn[:], in0=at[:], in1=rec[:], op=MUL)
    nc.sync.dma_start(attnT_d[128 * h:128 * (h + 1), t0:t0 + SC],
                      atn[:].bitcast(F32))


def _phase2(nc, tc, ctx, env, mode):
    mybir = env["mybir"]
    F32R = mybir.dt.float32r
    pools = {}
    pools["wqp"] = ctx.enter_context(tc.tile_pool(name="wqp", bufs=1))
    pools["ht2"] = ctx.enter_context(tc.tile_pool(name="ht2", bufs=4))
    pools["cs2"] = ctx.enter_context(tc.tile_pool(name="cs2", bufs=2))
    pools["tmp2"] = ctx.enter_context(tc.tile_pool(name="tmp2", bufs=2))
    pools["qtp"] = ctx.enter_context(tc.tile_pool(name="qtp", bufs=6))
    pools["prb"] = ctx.enter_context(tc.tile_pool(name="prb", bufs=1))
    pools["ans"] = ctx.enter_context(tc.tile_pool(name="ans", bufs=3))
    pools["rcp"] = ctx.enter_context(tc.tile_pool(name="rcp", bufs=2))
    pools["bia"] = ctx.enter_context(tc.tile_pool(name="bia", bufs=2))
    pools["qps"] = ctx.enter_context(tc.tile_pool(name="qps", bufs=2, space="PSUM"))
    pools["scps"] = ctx.enter_context(tc.tile_pool(name="scps", bufs=1, space="PSUM"))
    pools["atps"] = ctx.enter_context(tc.tile_pool(name="atps", bufs=1, space="PSUM"))
    pools["dnps"] = ctx.enter_context(tc.tile_pool(name="dnps", bufs=1, space="PSUM"))

    wq_all = pools["wqp"].tile([128, NKC * 512], F32R, tag="wq")
    nc.sync.dma_start(
        wq_all[:].rearrange("p (kk c) -> p kk c", c=512),
        env["wq"][:].bitcast(F32R).rearrange("(kk p) c -> p kk c", p=128))
    env["wq_all"] = wq_all

    for b in range(B):
        for kappa in range(NSC):
            qt = _qproj_rope(nc, pools, env, b, kappa)
            for h in range(HQ):
                _attn_head(nc, pools, env, mode, b, kappa, h, qt[h])


def _phase3(nc, tc, ctx, env):
    mybir = env["mybir"]
    F32, F32R = mybir.dt.float32, mybir.dt.float32r
    wo, attnT_d, out = env["wo"], env["attnT_d"], env["out"]
    wop = ctx.enter_context(tc.tile_pool(name="wop", bufs=1))
    atl = ctx.enter_context(tc.tile_pool(name="atl", bufs=2))
    osb = ctx.enter_context(tc.tile_pool(name="osb", bufs=4))
    ops = ctx.enter_context(tc.tile_pool(name="ops", bufs=6, space="PSUM"))

    wo_all = wop.tile([128, HQ * HID], F32R, tag="wo")
    nc.sync.dma_start(
        wo_all[:].rearrange("p (h c) -> p h c", c=HID),
        wo[:].bitcast(F32R).rearrange("(h p) c -> p h c", p=128))
    for g in range(TOK // SC):
        a_h = []
        for h in range(HQ):
            a = atl.tile([128, SC], F32R, tag=f"a{h}")
            nc.sync.dma_start(
                a[:], attnT_d[128 * h:128 * (h + 1),
                              SC * g:SC * (g + 1)].bitcast(F32R))
            a_h.append(a)
        for m in range(SC // 128):
            for n in range(HID // 512):
                ps = ops.tile([128, 512], F32, tag="o")
                for h in range(HQ):
                    nc.tensor.matmul(
                        ps[:], a_h[h][:, 128 * m:128 * (m + 1)],
                        wo_all[:, HID * h + 512 * n:HID * h + 512 * (n + 1)],
                        start=(h == 0), stop=(h == HQ - 1))
                ob = osb.tile([128, 512], F32, tag="ob")
                nc.any.tensor_copy(ob[:], ps[:])
                nc.sync.dma_start(
                    out[SC * g + 128 * m:SC * g + 128 * (m + 1),
                        512 * n:512 * (n + 1)], ob[:])


def _build_nc(mode, repeat=1):
    """mode in {"causal", "full", "bias"}; repeat>1 re-runs the whole kernel
    body for slope-based wall-clock timing."""
    import concourse.bass as bass  # noqa: F401
    import concourse.mybir as mybir
    import concourse.tile as tile
    from concourse import bacc

    F32 = mybir.dt.float32
    F32R = mybir.dt.float32r

    nc = bacc.Bacc("TRN2", target_bir_lowering=False)

    env = {"mybir": mybir}
    env["hT"] = nc.dram_tensor("hT", [HID, TOK], F32, kind="ExternalInput")
    env["wq"] = nc.dram_tensor("wq", [HID, HQ * D], F32, kind="ExternalInput")
    env["wk"] = nc.dram_tensor("wk", [HID, D], F32, kind="ExternalInput")
    env["wv"] = nc.dram_tensor("wv", [HID, D], F32, kind="ExternalInput")
    env["wo"] = nc.dram_tensor("wo", [HQ * D, HID], F32, kind="ExternalInput")
    env["cosT"] = nc.dram_tensor("cosT", [D, TOK], F32, kind="ExternalInput")
    env["sinR"] = nc.dram_tensor("sinR", [D, TOK], F32, kind="ExternalInput")
    rotm = nc.dram_tensor("rotm", [128, 128], F32, kind="ExternalInput")
    ident = nc.dram_tensor("ident", [128, 128], F32, kind="ExternalInput")
    ones = nc.dram_tensor("ones", [128, 128], F32, kind="ExternalInput")
    mbig = nc.dram_tensor("mbig", [128, 896], F32, kind="ExternalInput")
    if mode == "bias":
        env["biasT"] = nc.dram_tensor("biasT", [B, S, S], F32, kind="ExternalInput")
    env["out"] = nc.dram_tensor("out", [TOK, HID], F32, kind="ExternalOutput")

    with tile.TileContext(nc) as tc, ExitStack() as ctx:
        cpool = ctx.enter_context(tc.tile_pool(name="consts", bufs=1))
        kvsb = ctx.enter_context(tc.tile_pool(name="kvsb", bufs=1))
        adp = ctx.enter_context(tc.tile_pool(name="adram", bufs=1, space="DRAM"))

        env["rotm_t"] = cpool.tile([128, 128], F32R, tag="rotm", name="rotm_t")
        env["ident_t"] = cpool.tile([128, 128], F32, tag="ident", name="ident_t")
        env["ones_t"] = cpool.tile([128, 128], F32R, tag="ones", name="ones_t")
        env["mbig_t"] = cpool.tile([128, 896], F32, tag="mbig", name="mbig_t")
        nc.sync.dma_start(env["rotm_t"][:], rotm[:].bitcast(F32R))
        nc.sync.dma_start(env["ident_t"][:], ident[:])
        nc.sync.dma_start(env["ones_t"][:], ones[:].bitcast(F32R))
        nc.sync.dma_start(env["mbig_t"][:], mbig[:])

        env["attnT_d"] = adp.tile([HQ * D, TOK], F32, tag="attnTd", name="attnT_d")
        env["kt_b"] = [kvsb.tile([128, S], F32R, tag=f"ktb{b}", name=f"kt_b{b}") for b in range(B)]
        env["vtm_b"] = [kvsb.tile([128, S], F32R, tag=f"vtmb{b}", name=f"vtm_b{b}") for b in range(B)]

        for _rep in range(repeat):
            with ExitStack() as p1ctx:
                _phase1(nc, tc, p1ctx, env)
            with ExitStack() as p2ctx:
                _phase2(nc, tc, p2ctx, env, mode)
            with ExitStack() as p3ctx:
                _phase3(nc, tc, p3ctx, env)
    nc.finalize()
    return nc


def _get_runner(mode):
    if mode in _RUNNERS:
        return _RUNNERS[mode]
    nc = _build_nc(mode)
    _RUNNERS[mode] = nc
    return nc


def _host_prep(hidden_states, Wq, Wk, Wv, Wo, cos_cache, sin_cache,
               position_ids, attention_mask):
    hidden_states = np.asarray(hidden_states, dtype=np.float32)
    Wq = np.asarray(Wq, dtype=np.float32)
    Wk = np.asarray(Wk, dtype=np.float32)
    Wv = np.asarray(Wv, dtype=np.float32)
    Wo = np.asarray(Wo, dtype=np.float32)
    cos_cache = np.asarray(cos_cache, dtype=np.float32)
    sin_cache = np.asarray(sin_cache, dtype=np.float32)
    position_ids = np.asarray(position_ids)
    mask = np.asarray(attention_mask)

    hT = np.ascontiguousarray(hidden_states.reshape(TOK, HID).T)
    cos_g = cos_cache[position_ids.astype(np.int64)]   # [B, S, D]
    sin_g = sin_cache[position_ids.astype(np.int64)]
    cosT = np.ascontiguousarray(cos_g.reshape(TOK, D).T)          # [D, TOK]
    sinT = np.ascontiguousarray(sin_g.reshape(TOK, D).T)
    sinR = np.ascontiguousarray(np.roll(sinT, -64, axis=0))       # w[d]=sin[(d+64)%128]

    m2 = mask.reshape(B, S, S)
    tril = np.tril(np.ones((S, S), dtype=bool))
    if all(np.array_equal(m2[b], tril) for b in range(B)):
        mode = "causal"
    elif m2.all():
        mode = "full"
    else:
        mode = "bias"

    rotm = np.zeros((128, 128), dtype=np.float32)
    for i in range(64):
        rotm[64 + i, i] = -1.0
        rotm[i, 64 + i] = 1.0
    identm = np.eye(128, dtype=np.float32)
    onesm = np.ones((128, 128), dtype=np.float32)
    # Mbig[p, y] = 1 iff y >= p + 384  (slices give the 4 diagonal masks)
    yy = np.arange(896)[None, :]
    pp = np.arange(128)[:, None]
    mbig = (yy >= pp + 384).astype(np.float32)

    scale = np.float32(1.0 / math.sqrt(D))
    common = dict(hT=hT, cosT=cosT, sinR=sinR, rotm=rotm, ident=identm,
                  ones=onesm, mbig=mbig)
    if mode == "bias":
        biasT = np.where(m2, np.float32(0), np.float32(-1e30)).astype(np.float32)
        biasT = np.ascontiguousarray(biasT.transpose(0, 2, 1))  # [B, t, s]
        common["biasT"] = biasT

    in_maps = []
    for c in range(NCORES):
        m = dict(common)
        m["wq"] = np.ascontiguousarray(Wq[:, c * HQ * D:(c + 1) * HQ * D] * scale)
        m["wk"] = np.ascontiguousarray(Wk[:, c * D:(c + 1) * D])
        m["wv"] = np.ascontiguousarray(Wv[:, c * D:(c + 1) * D])
        m["wo"] = np.ascontiguousarray(Wo[c * HQ * D:(c + 1) * HQ * D, :])
        in_maps.append(m)
    return mode, in_maps


def kernel(hidden_states, Wq, Wk, Wv, Wo, cos_cache, sin_cache,
           position_ids, attention_mask):
    from concourse.bass_utils import run_bass_kernel_spmd

    mode, in_maps = _host_prep(hidden_states, Wq, Wk, Wv, Wo, cos_cache,
                               sin_cache, position_ids, attention_mask)
    nc = _get_runner(mode)
    res = run_bass_kernel_spmd(nc, in_maps, core_ids=list(range(NCORES)),
                               trace=False)
    acc = np.zeros((TOK, HID), dtype=np.float32)
    for c in range(NCORES):
        acc += res.results[c]["out"]
    return acc.reshape(B, S, HID)
